# revision 27
# baseline (speedup 1.0000x reference)
"""Trainium2 Bass kernel for AttentionUpscaling (sparse attention rescoring).

Math (reference):
  hf_flat[b,n,:]  = hr_hf_patches[b,:,h,w]    (n = h*nw + w)   -- (B,N,D) D=1024
  base_flat       = same for base_hf_patches
  key_emb = pool+linear(hf)  = hf_flat @ Weff_k + bk           -- (B,N,E) E=128
  q_emb   = base_flat @ Weff_q + bq        (Weff = A_pool^T @ W, pooling is linear)
  prior, idx = top16(hr_attn[b,n,:])
  pair MLP: h = gelu(q@W1q + k@W1k + (q-k)@W1d + (q*k)@W1p + prior*w1p + b1)
          = gelu(q@(W1q+W1d) + k@(W1k-W1d) + (q*k)@W1p + prior*w1p + b1)
  resid = h@W2 + b2 ;  w = softmax(log(max(prior,1e-8)) + resid)
  out[b,n,:] = sum_k w_k * hf_flat[b, idx_k, :]

Sharding: queries (N) split across 8 cores; key tables replicated per core.

v2: single extended-row gather. Host stages ext[b] = [NK, 1152] fp16 rows
  [hf_row (1024) | kemb slot (128)]; the kernel writes kemb into the slot
  during encode, then ONE SWDGE gather per 1024 pairs fetches both the MLP
  key embedding and the HF row (2304B descriptors) -- half the descriptor
  count of the v1 two-gather scheme. kpack^T for the rescore matmuls is
  recovered with PE transposes of the gathered kemb chunks. The rank-1
  prior term moved from a contract-1 PE matmul to a DVE STT against a
  partition-broadcast prior row.
"""

import os
import sys
import math
import numpy as np

sys.path.insert(0, "/opt/trn_rl_repo")

try:  # make the NTFF profile hook shim importable as antenv.axon_hooks
    import antenv

    _p = "/opt/trn_rl_repo/antenv"
    if os.path.isdir(_p) and _p not in list(antenv.__path__):
        antenv.__path__.append(_p)
except Exception:
    pass

import concourse.bass as bass
import concourse.bacc as bacc
import concourse.hw_specs as hw_specs

# Feed the Tile scheduler measured SWDGE gather numbers (fixed ~0.9us +
# ~7.6ns/descriptor on silicon) so the static schedule overlaps them.
hw_specs.TRN2Spec.SWDGE_FIXED_OVERHEAD_NS = 900
hw_specs.TRN2Spec.SWDGE_NS_PER_DESCRIPTOR = 7.6
import concourse.mybir as mybir
import concourse.tile as tile
from concourse.bass_utils import run_bass_kernel_spmd

dt = mybir.dt
AF = mybir.ActivationFunctionType
ALU = mybir.AluOpType

STEM_C = 16
POOL = 4
P = 8
ROW = 1152  # extended row: 1024 hf + 128 kemb


class Cfg:
    def __init__(self, nq=512, nk=4096, ncores=8):
        self.B = 2
        self.D = 1024
        self.E = 128
        self.H = 64
        self.K = 16
        self.din = STEM_C * POOL * POOL  # 256
        self.ncores = ncores
        self.nq = nq            # queries per core per batch
        self.nk = nk            # total keys (= N)
        self.pairs = nq * self.K
        self.nt = nq // 128     # topk tiles per batch
        assert nq % 128 == 0 and self.pairs % 1024 == 0


def build_nc(cfg: Cfg, debug=False):
    B, D, E, H, K = cfg.B, cfg.D, cfg.E, cfg.H, cfg.K
    NQ, NK = cfg.nq, cfg.nk
    f32, f16, u16, i16 = dt.float32, dt.float16, dt.uint16, dt.int16

    nc = bacc.Bacc("TRN2", target_bir_lowering=False, debug=debug,
                   num_devices=cfg.ncores)

    # ---------------- DRAM parameters ----------------
    attn = nc.dram_tensor("attn", [B, NQ, NK], f32, kind="ExternalInput").ap()
    base_dm = nc.dram_tensor("base_dm16", [B, D, NQ], f16, kind="ExternalInput").ap()
    hfk_dm = nc.dram_tensor("hf_dm16", [B, D, NK], f16, kind="ExternalInput").ap()
    # one ext table per batch so batch-0 gathers only wait on batch-0 encode
    exts = [nc.dram_tensor(f"exthf{b}", [NK, ROW], f16, kind="ExternalInput").ap()
            for b in range(B)]
    wq_d = nc.dram_tensor("wq", [cfg.din, E], f32, kind="ExternalInput").ap()
    wk_d = nc.dram_tensor("wk", [cfg.din, E], f32, kind="ExternalInput").ap()
    w1_d = nc.dram_tensor("w1", [4 * E + 1, H], f32, kind="ExternalInput").ap()
    w2_d = nc.dram_tensor("w2", [H, 1], f32, kind="ExternalInput").ap()
    bq_d = nc.dram_tensor("bq", [E, 1], f32, kind="ExternalInput").ap()
    bk_d = nc.dram_tensor("bk", [E, 1], f32, kind="ExternalInput").ap()
    b1_d = nc.dram_tensor("b1", [H, 1], f32, kind="ExternalInput").ap()
    b2_d = nc.dram_tensor("b2", [1, 1], f32, kind="ExternalInput").ap()
    apool_d = nc.dram_tensor("apool", [cfg.din, D], f32, kind="ExternalInput").ap()
    mask_d = nc.dram_tensor("maskblk", [128, 8, 64], f32, kind="ExternalInput").ap()
    ident_d = nc.dram_tensor("ident16", [128, 128], f16, kind="ExternalInput").ap()
    out_d = nc.dram_tensor("out", [B, NQ, D], f32, kind="ExternalOutput").ap()

    with tile.TileContext(nc) as tc:
        with (
            tc.tile_pool(name="const", bufs=1) as constp,
            tc.tile_pool(name="dram", bufs=1, space="DRAM") as dramp,
            tc.tile_pool(name="psA", bufs=2, space="PSUM") as psA,
            tc.tile_pool(name="psB", bufs=2, space="PSUM") as psB,
            tc.tile_pool(name="psO", bufs=2, space="PSUM") as psO,
        ):
            # ================= init: weights =================
            initp = tc.alloc_tile_pool(name="init", bufs=1)
            wq_sb = initp.tile([128, 2, E], f32)
            wk_sb = initp.tile([128, 2, E], f32)
            nc.sync.dma_start(wq_sb[:], wq_d.rearrange("(c p) e -> p c e", p=128))
            nc.sync.dma_start(wk_sb[:], wk_d.rearrange("(c p) e -> p c e", p=128))
            apool_sb = initp.tile([128, 2, D], f32)
            nc.sync.dma_start(apool_sb[:], apool_d.rearrange("(c p) d -> p c d", p=128))
            mask_sb = constp.tile([128, 8, 64], f32)
            nc.sync.dma_start(mask_sb[:], mask_d)
            ident16 = constp.tile([128, 128], f16)
            nc.sync.dma_start(ident16[:], ident_d)
            bq_sb = constp.tile([E, 1], f32)
            bk_sb = constp.tile([E, 1], f32)
            b1_sb = constp.tile([H, 1], f32)
            b2_sb = constp.tile([1, 1], f32)
            for dst, src in ((bq_sb, bq_d), (bk_sb, bk_d), (b1_sb, b1_d), (b2_sb, b2_d)):
                nc.sync.dma_start(dst[:], src)

            # W1 pieces: rows [0:128]=q, [128:256]=k, [256:384]=d, [384:512]=p, [512]=prior
            w1_sb = initp.tile([128, 4, H], f32)
            nc.sync.dma_start(w1_sb[:], w1_d[0:512, :].rearrange("(c p) h -> p c h", p=128))
            w1pr_sb = constp.tile([1, H], f32)
            nc.sync.dma_start(w1pr_sb[:], w1_d[512:513, :])
            w1pr16 = constp.tile([1, H], f16)
            nc.vector.tensor_copy(w1pr16[:], w1pr_sb[:])
            w1qp = constp.tile([128, H], f16)
            w1kp = constp.tile([128, H], f16)
            w1p = constp.tile([128, H], f16)
            nc.vector.tensor_add(w1qp[:], w1_sb[:, 0, :], w1_sb[:, 2, :])
            nc.vector.tensor_sub(w1kp[:], w1_sb[:, 1, :], w1_sb[:, 2, :])
            nc.vector.tensor_copy(w1p[:], w1_sb[:, 3, :])
            w2_sb = initp.tile([H, 1], f32)
            nc.sync.dma_start(w2_sb[:], w2_d)
            w2_16 = constp.tile([H, 1], f16)
            nc.vector.tensor_copy(w2_16[:], w2_sb[:])

            # Weff = A_pool^T @ W  -> stored as 8 chunks of (128 D-rows, E), fp16
            weffq = constp.tile([128, 8, E], f16)
            weffk = constp.tile([128, 8, E], f16)
            for wsb, weff in ((wq_sb, weffq), (wk_sb, weffk)):
                for r in range(8):
                    ps_w = psA.tile([128, 512], f32, tag="psA")
                    for k2 in range(2):
                        nc.tensor.matmul(ps_w[:, 0:E], apool_sb[:, k2, r * 128:(r + 1) * 128],
                                         wsb[:, k2, :], start=(k2 == 0), stop=(k2 == 1))
                    nc.scalar.activation(weff[:, r, :], ps_w[:, 0:E], AF.Copy)

            initp.release()
            encp = tc.alloc_tile_pool(name="enc", bufs=2)
            attnp = tc.alloc_tile_pool(name="attn_pool", bufs=3)
            smallp = tc.alloc_tile_pool(name="small", bufs=1)
            kgp = tc.alloc_tile_pool(name="kg_pool", bufs=5)
            ccp = tc.alloc_tile_pool(name="cc", bufs=3)
            outp = tc.alloc_tile_pool(name="outp", bufs=2)

            # ============ encode emission units ============
            qp = tc.alloc_tile_pool(name="qpool", bufs=1)
            qts = [None, None]

            def emit_enc_q(b):
                bsb = encp.tile([128, 8, 512], f16, tag="encrhs")
                nc.gpsimd.dma_start(bsb[:, :, 0:NQ], base_dm[b].rearrange("(c p) n -> p c n", p=128))
                ps_q = psA.tile([128, 512], f32, tag="psA")
                for k2 in range(8):
                    nc.tensor.matmul(ps_q[:, 0:NQ], weffq[:, k2, :], bsb[:, k2, 0:NQ],
                                     start=(k2 == 0), stop=(k2 == 7))
                qT16 = qp.tile([E, 512], f16, tag=f"qT16_{b}")
                nc.scalar.activation(qT16[:, 0:NQ], ps_q[:, 0:NQ], AF.Identity, bias=bq_sb[:, 0:1])
                qts[b] = qT16

            def emit_enc_k(b, kc):
                # keys: kemb written into ext rows via PE transpose
                # (rows [n, 1024:1152] of the extended table)
                ksb = encp.tile([128, 8, 512], f16, tag="encrhs")
                nc.gpsimd.dma_start(
                    ksb[:], hfk_dm[b, :, kc * 512:(kc + 1) * 512]
                    .rearrange("(c p) n -> p c n", p=128))
                ps_k = psA.tile([128, 512], f32, tag="psA")
                for k2 in range(8):
                    nc.tensor.matmul(ps_k[:], weffk[:, k2, :], ksb[:, k2, :],
                                     start=(k2 == 0), stop=(k2 == 7))
                kT16 = encp.tile([E, 512], f16, tag="kT16")
                nc.scalar.activation(kT16[:], ps_k[:], AF.Identity, bias=bk_sb[:, 0:1])
                kcat_sb = smallp.tile([128, 4, E], f16, tag="kcat_sb", bufs=3)
                for tt in range(4):
                    sl = slice(tt * 128, (tt + 1) * 128)
                    ps_t1 = psA.tile([128, 512], f16, tag="psA")
                    nc.tensor.transpose(ps_t1[:, 0:128], kT16[:, sl], ident16[:])
                    nc.scalar.activation(kcat_sb[:, tt, :], ps_t1[:, 0:128], AF.Copy)
                nc.sync.dma_start(
                    exts[b][kc * 512:(kc + 1) * 512, 1024:1152]
                    .rearrange("(tt p) e -> p tt e", p=128),
                    kcat_sb[:])

            # ============ 3 attn prefetches ahead of the encode loads ====
            tiles = [(b, t) for b in range(B) for t in range(cfg.nt)]
            attn_pre = {}

            def emit_attn_load(s):
                b, t = tiles[s]
                asb = attnp.tile([128, NK], f32, tag="attn_t", name=f"attn_{b}_{t}")
                nc.sync.dma_start(asb[:], attn[b, t * 128:(t + 1) * 128, :])
                attn_pre[s] = asb

            for s in range(3):
                emit_attn_load(s)

            # encode both batches up front (replicated per core)
            emit_enc_q(0)
            emit_enc_q(1)
            for kc in range(NK // 512):
                emit_enc_k(0, kc)
            for kc in range(NK // 512):
                emit_enc_k(1, kc)

            # ============ software-pipelined tile loop ============
            st = {}

            def emit_topk(s):
                b, t = tiles[s]
                if s in attn_pre:
                    asb = attn_pre.pop(s)
                else:
                    asb = attnp.tile([128, NK], f32, tag="attn_t", name=f"attn_{b}_{t}")
                    nc.sync.dma_start(asb[:], attn[b, t * 128:(t + 1) * 128, :])
                idx_t = smallp.tile([128, K], u16, tag="idx_t", bufs=8, name=f"idx_{b}_{t}")
                prior_t = smallp.tile([128, K], f32, tag="prior_t", bufs=8, name=f"prior_{b}_{t}")
                nc.vector.max(prior_t[:, 0:8], asb[:])
                nc.vector.max_index(idx_t[:, 0:8], prior_t[:, 0:8], asb[:])
                nc.vector.match_replace(asb[:], prior_t[:, 0:8], asb[:], -1e30)
                nc.vector.max(prior_t[:, 8:16], asb[:])
                nc.vector.max_index(idx_t[:, 8:16], prior_t[:, 8:16], asb[:])
                pcl_t = smallp.tile([128, K], f32, tag="pcl_t", bufs=8, name=f"pcl_{b}_{t}")
                nc.vector.tensor_scalar_max(pcl_t[:], prior_t[:], 1e-8)
                prior16 = smallp.tile([128, K], f16, tag="prior16", bufs=8,
                                      name=f"pr16_{b}_{t}")
                nc.scalar.activation(prior16[:], prior_t[:], AF.Copy)
                idx_scr = dramp.tile([K, 128], u16, name=f"idx_scr{b}_{t}")
                nc.scalar.dma_start(idx_scr[:].rearrange("kk qq -> qq kk"), idx_t[:])
                pr_scr = dramp.tile([2048], f16, name=f"pr_scr{b}_{t}")
                nc.scalar.dma_start(
                    pr_scr[:].rearrange("(qq kk) -> qq kk", kk=K), prior16[:])
                idxp1 = smallp.tile([128, 128], u16, tag="idxp1", bufs=8,
                                    name=f"idxp{b}_{t}")
                nc.scalar.dma_start(
                    idxp1[:],
                    idx_scr[:].unsqueeze(0).broadcast_to((8, K, 128)),
                )
                return dict(pcl_t=pcl_t, idxp1=idxp1, pr_scr=pr_scr)

            def emit_gather(s):
                b, t = tiles[s]
                S = st[s]
                idxp1 = S["idxp1"]
                kgs = []
                for g2 in range(2):
                    kg = kgp.tile([128, 8, ROW], f16, tag="kg",
                                  name=f"kg{b}_{t}_{g2}")
                    nc.gpsimd.dma_gather(
                        kg[:], exts[b][:],
                        idxp1[:, g2 * 64:(g2 + 1) * 64].bitcast(i16),
                        1024, 1024, ROW, transpose=False,
                    )
                    kgs.append(kg)
                S["kgs"] = kgs

            def emit_rescore(s):
                b, t = tiles[s]
                S = st[s]
                pcl_t, pr_scr, kgs = S["pcl_t"], S["pr_scr"], S["kgs"]
                qT16 = qts[b]
                resid_scr = dramp.tile([2048], f32, name=f"resid_scr{b}_{t}")
                # prior row [1, 2048] for the rank-1 prior matmul
                priort_row = smallp.tile([1, 2048], f16, tag="priort_row", bufs=3,
                                         name=f"priorrow{b}_{t}")
                nc.scalar.dma_start(priort_row[:], pr_scr[:])
                for hh2 in range(4):
                    kg = kgs[hh2 // 2]
                    j0 = (hh2 % 2) * 4
                    # kpack^T [E, 512] via 4 PE transposes of kemb chunks
                    ps_t = psA.tile([128, 512], f16, tag="psA")
                    for jj in range(4):
                        nc.tensor.transpose(ps_t[:, jj * 128:(jj + 1) * 128],
                                            kg[:, j0 + jj, 1024:1152], ident16[:])
                    kpackT = ccp.tile([E, 512], f16, tag="kpackT")
                    nc.scalar.activation(kpackT[:], ps_t[:], AF.Copy)
                    nq0 = t * 128 + hh2 * 32
                    qrep_ap = (qT16[:, nq0:nq0 + 32].unsqueeze(2)
                               .broadcast_to((E, 32, 16)))
                    prod = ccp.tile([E, 512], f16, tag="prod")
                    nc.vector.tensor_mul(
                        prod[:].rearrange("p (n j) -> p n j", j=16),
                        kpackT[:].rearrange("p (n j) -> p n j", j=16),
                        qrep_ap)
                    # full MLP preactivation accumulated in PSUM; bias+gelu
                    # fused into one ACT op (no DVE adds at all)
                    ps_h = psB.tile([128, 512], f32, tag="psB")
                    nc.tensor.matmul(ps_h[0:H, :], w1kp[:], kpackT[:],
                                     start=True, stop=False)
                    nc.tensor.matmul(
                        ps_h[0:H, :].rearrange("p (n j) -> p n j", j=16),
                        w1qp[:], qrep_ap, start=False, stop=False)
                    nc.tensor.matmul(ps_h[0:H, :], w1pr16[:],
                                     priort_row[:, hh2 * 512:(hh2 + 1) * 512],
                                     start=False, stop=False)
                    nc.tensor.matmul(ps_h[0:H, :], w1p[:], prod[:],
                                     start=False, stop=True)
                    h16 = ccp.tile([H, 512], f16, tag="h16")
                    nc.scalar.activation(h16[:], ps_h[0:H, :], AF.Gelu_apprx_tanh,
                                         bias=b1_sb[:, 0:1])
                    ps_r = psA.tile([128, 512], f32, tag="psA")
                    nc.tensor.matmul(ps_r[0:1, :], w2_16[:], h16[:])
                    residc = ccp.tile([1, 512], f32, tag="residc")
                    nc.vector.tensor_scalar_add(residc[:], ps_r[0:1, :], b2_sb[0:1, 0:1])
                    nc.scalar.dma_start(resid_scr[hh2 * 512:(hh2 + 1) * 512], residc[:])
                # softmax
                residq = smallp.tile([128, K], f32, tag="residq", bufs=2)
                nc.scalar.dma_start(
                    residq[:], resid_scr[:].rearrange("(qq kk) -> qq kk", kk=K))
                wexp = smallp.tile([128, K], f32, tag="wexp", bufs=2)
                nc.scalar.activation(wexp[:], residq[:], AF.Exp)
                wun = smallp.tile([128, K], f32, tag="wun", bufs=2)
                ssum = smallp.tile([128, 1], f32, tag="ssum", bufs=2)
                nc.vector.scalar_tensor_tensor(wun[:], wexp[:], 1.0, pcl_t[:],
                                               ALU.mult, ALU.mult, accum_out=ssum[:])
                rs = smallp.tile([128, 1], f32, tag="rs", bufs=2)
                nc.vector.reciprocal(rs[:], ssum[:])
                wnorm = smallp.tile([128, K], f32, tag="wnorm", bufs=2)
                nc.vector.tensor_tensor(wnorm[:], wun[:],
                                        rs[:].broadcast_to((128, K)), ALU.mult)
                wn_scr = dramp.tile([2048], f32, name=f"wn_scr{b}_{t}")
                nc.scalar.dma_start(
                    wn_scr[:].rearrange("(qq kk) -> qq kk", kk=K), wnorm[:])
                wpair = smallp.tile([128, 16, 1], f32, tag="wpair", bufs=2)
                nc.scalar.dma_start(
                    wpair[:, :, 0], wn_scr[:].rearrange("(blk p) -> p blk", p=128))
                wblk_t = smallp.tile([128, 16, 64], f16, tag="wblk", bufs=2,
                                     name=f"wblk{b}_{t}")
                nc.vector.scalar_tensor_tensor(
                    wblk_t[:].rearrange("p (gm j) q -> p gm j q", j=8),
                    wpair[:].rearrange("p (gm j) one -> p gm j one", j=8)
                        .broadcast_to((128, 2, 8, 64)),
                    1.0,
                    mask_sb[:].unsqueeze(1).broadcast_to((128, 2, 8, 64)),
                    ALU.mult, ALU.mult,
                )
                S["wblk_t"] = wblk_t

            def emit_wsum(s):
                b, t = tiles[s]
                S = st[s]
                wblk_t, kgs = S["wblk_t"], S["kgs"]
                ps_o = psO.tile([128, D], f32, tag="psO")
                for g2 in range(2):
                    kg = kgs[g2]
                    base = 64 * g2
                    for csl in (slice(0, 512), slice(512, D)):
                        for j in range(8):
                            nc.tensor.matmul(
                                ps_o[base:base + 64, csl],
                                wblk_t[:, g2 * 8 + j, :],
                                kg[:, j, csl],
                                start=(j == 0), stop=(j == 7),
                            )
                for csl in (slice(0, 512), slice(512, D)):
                    osb = outp.tile([128, 512], f32, tag="osb")
                    nc.scalar.activation(osb[:], ps_o[:, csl], AF.Copy)
                    nc.sync.dma_start(out_d[b, t * 128:(t + 1) * 128, csl], osb[:])

            # topk for all tiles up front (DVE backbone), then gathers,
            # then rescore+wsum -- each engine queue stays dependency-clean
            NTILES = len(tiles)
            for s in range(NTILES):
                if s + 3 < NTILES:
                    emit_attn_load(s + 3)
                st[s] = emit_topk(s)
            for s in range(NTILES):
                emit_gather(s)
            for s in range(NTILES):
                emit_rescore(s)
                emit_wsum(s)

            for p_ in (qp, outp, ccp, kgp, smallp, attnp, encp):
                p_.release()

    nc.compile()
    return nc


# ---------------------------------------------------------------------------
# Host side
# ---------------------------------------------------------------------------

def _make_apool():
    A = np.zeros((STEM_C * POOL * POOL, STEM_C * P * P), np.float32)
    s = P // POOL
    for c in range(STEM_C):
        for py in range(POOL):
            for px in range(POOL):
                o = (c * POOL + py) * POOL + px
                for dy in range(s):
                    for dx in range(s):
                        d = (c * P + py * s + dy) * P + px * s + dx
                        A[o, d] = 1.0 / (s * s)
    return A


def make_in_maps(inputs, cfg: Cfg):
    B, D = cfg.B, cfg.D
    NQ, NK, NC = cfg.nq, cfg.nk, cfg.ncores
    hr_attn = np.asarray(inputs["hr_attn"], np.float32)
    hr_hf = np.asarray(inputs["hr_hf_patches"], np.float32).reshape(B, D, NK)
    base_hf = np.asarray(inputs["base_hf_patches"], np.float32).reshape(B, D, NK)
    ext = np.zeros((B, NK, ROW), np.float16)
    ext[:, :, 0:D] = hr_hf.transpose(0, 2, 1).astype(np.float16)
    ext = np.ascontiguousarray(ext)

    common = dict(
        wq=np.asarray(inputs["Wq"], np.float32),
        wk=np.asarray(inputs["Wk"], np.float32),
        w1=np.asarray(inputs["W1"], np.float32),
        w2=np.asarray(inputs["W2"], np.float32).reshape(cfg.H, 1),
        bq=np.asarray(inputs["bq"], np.float32).reshape(cfg.E, 1),
        bk=np.asarray(inputs["bk"], np.float32).reshape(cfg.E, 1),
        b1=np.asarray(inputs["b1"], np.float32).reshape(cfg.H, 1),
        b2=np.asarray(inputs["b2"], np.float32).reshape(1, 1),
        apool=_make_apool(),
        maskblk=np.equal(np.arange(64)[None, None, :], 8 * np.arange(8)[None, :, None] + (np.arange(128) // 16)[:, None, None]).astype(np.float32),
        ident16=np.eye(128, dtype=np.float16),
        exthf0=ext[0],
        exthf1=ext[1],
    )
    common["hf_dm16"] = hr_hf.astype(np.float16)
    in_maps = []
    for c in range(NC):
        sl = slice(c * NQ, (c + 1) * NQ)
        m = dict(common)
        m["attn"] = np.ascontiguousarray(hr_attn[:, sl, :])
        m["base_dm16"] = np.ascontiguousarray(base_hf[:, :, sl]).astype(np.float16)
        in_maps.append(m)
    return in_maps


_NC_CACHE = {}


def _get_nc(cfg: Cfg):
    key = (cfg.nq, cfg.nk, cfg.ncores)
    if key not in _NC_CACHE:
        _NC_CACHE[key] = build_nc(cfg)
    return _NC_CACHE[key]


def run(inputs, trace=False, cfg=None, dbg=False):
    cfg = cfg or Cfg()
    nc = _get_nc(cfg)
    in_maps = make_in_maps(inputs, cfg)
    res = run_bass_kernel_spmd(nc, in_maps, core_ids=list(range(cfg.ncores)),
                               trace=trace)
    B, D, NQ, NC = cfg.B, cfg.D, cfg.nq, cfg.ncores
    out = np.empty((B, NC * NQ, D), np.float32)
    for c in range(NC):
        out[:, c * NQ:(c + 1) * NQ, :] = res.results[c]["out"]
    return out, res


def kernel(**inputs) -> np.ndarray:
    tk = inputs.get("topk", 16)
    assert int(np.asarray(tk)) == 16, "kernel is specialized for topk=16"
    out, res = run(inputs, trace=bool(os.environ.get("BASS_KERNEL_TRACE")))
    if res.exec_time_ns is not None:
        print(f"HW exec time: {res.exec_time_ns} ns")
    return out
